# revision 1
# baseline (speedup 1.0000x reference)
"""Single-head causal attention (B=4, T=4096, C=1024, H=64) on 8 TRN2 cores.

Sharding: 2 cores per batch element, query rows split between the pair so
causal work is balanced. Fold 0 owns query 512-blocks starting at
{512, 1024, 2048, 3584}; fold 1 owns {0, 1536, 2560, 3072}. Grouped into 4
slots with uniform per-slot key-chunk bounds U = {8, 16, 24, 32} the SPMD
program is identical on both folds with only 8/72 wasted key-chunk
iterations per core. Causal masking is data-driven (query-index tensor vs
key indices compared on DVE), so per-core differences are input data only.

Numerics: all matmul operands bf16 with fp32 PSUM accumulation; softmax
needs no max-subtraction (|scores| <= |q||k|/8 ~ 2.6). A ones-column is
appended to v so the softmax denominator falls out of the same matmul.
Host passes x pre-transposed and pre-tiled for contiguous DMA.
"""

import numpy as np
import ml_dtypes

import concourse.bacc as bacc
import concourse.mybir as mybir
from concourse.tile import TileContext
from concourse.masks import make_identity
from concourse.bass_utils import run_bass_kernel_spmd

B, T, C, H = 4, 4096, 1024, 64
P = 128                     # SBUF partitions
NB = T // P                 # 32 key chunks of 128
CB = C // P                 # 8 contraction chunks of 128
QS = 512                    # query/projection block width
TB = T // QS                # 8 key-side projection blocks
NSLOT = 4                   # query slots per core (2048 queries)
HE = H + 1                  # v extended with a ones column (softmax denom)

FOLD_SLOT_QSTART = [
    [512, 1024, 2048, 3584],    # fold 0
    [0, 1536, 2560, 3072],      # fold 1
]
SLOT_U = [8, 16, 24, 32]        # key chunks per slot (uniform across folds)
# tks needing the data-driven causal mask (diagonal band of either fold)
SLOT_MASK_TK = [range(u - 8, u) for u in SLOT_U]

F32 = mybir.dt.float32
BF16 = mybir.dt.bfloat16
BF16NP = ml_dtypes.bfloat16


def build_bass():
    nc = bacc.Bacc("TRN2", target_bir_lowering=False, debug=False)

    x_kv_d = nc.declare_dram_parameter("x_kv", [TB, P, CB, QS], BF16, isOutput=False)
    x_q_d = nc.declare_dram_parameter("x_q", [NSLOT, P, CB, QS], BF16, isOutput=False)
    w_d = nc.declare_dram_parameter("w_all", [P, 3, CB, H], BF16, isOutput=False)
    b_d = nc.declare_dram_parameter("b_all", [H, 3], F32, isOutput=False)
    qkidx_d = nc.declare_dram_parameter(
        "qkidx", [P, NSLOT * QS + NB], F32, isOutput=False
    )
    out_d = nc.declare_dram_parameter("out", [NSLOT * QS, H], F32, isOutput=True)

    with TileContext(nc) as tc:
        with (
            tc.tile_pool(name="const", bufs=1) as const,
            tc.tile_pool(name="xio", bufs=4) as xio,
            tc.tile_pool(name="work", bufs=3) as work,
            tc.tile_pool(name="mpool", bufs=2) as mpool,
            tc.tile_pool(name="ps_s", bufs=2, space="PSUM") as ps_s,
            tc.tile_pool(name="ps_o", bufs=1, space="PSUM") as ps_o,
            tc.tile_pool(name="ps_p", bufs=2, space="PSUM") as ps_p,
            tc.tile_pool(name="ps_t", bufs=1, space="PSUM") as ps_t,
        ):
            # ---- persistent SBUF state ----
            w_sb = const.tile([P, 3, CB, H], BF16, tag="w")
            nc.sync.dma_start(w_sb[:], w_d[:])
            b_sb = const.tile([H, 3], F32, tag="b")
            nc.sync.dma_start(b_sb[:], b_d[:])
            qkidx_sb = const.tile([P, NSLOT * QS + NB], F32, tag="qkidx")
            nc.sync.dma_start(qkidx_sb[:], qkidx_d[:])
            qidx_sb = qkidx_sb[:, : NSLOT * QS]
            kidx_sb = qkidx_sb[:, NSLOT * QS :]

            id_f32 = const.tile([P, P], F32, tag="idf")
            id_bf16 = const.tile([P, P], BF16, tag="idb")
            make_identity(nc, id_f32[:])
            make_identity(nc, id_bf16[:])

            # kT/qT live twice: partitions 0-63 and a copy at 64-127 so the
            # two K=64 scores matmuls of a pair run in disjoint PE row-groups
            kT_sb = const.tile([P, T], BF16, tag="kT")           # [128, 4096]
            qT_sb = const.tile([P, NSLOT * QS], BF16, tag="qT")  # [128, 2048]
            vTb_sb = const.tile([H, T], BF16, tag="vTb")         # [64, 4096]
            vext_sb = const.tile([P, NB, HE], BF16, tag="vext")  # [128, 32, 65]
            nc.vector.memset(vext_sb[:, :, H:HE], 1.0)

            # ---- emission helpers (PE is in-order; emission order is the
            # static schedule). kv projections are split into small thunks so
            # they can be interleaved into the attention slots, filling the
            # PE bubbles left by the ACT exp latency. ----
            def kv_thunks(tb):
                st = {}
                cols = slice(tb * QS, (tb + 1) * QS)

                def load():
                    st["xt"] = xio.tile([P, CB, QS], BF16, tag="xt", name="xt")
                    nc.sync.dma_start(st["xt"][:], x_kv_d[tb])

                def mk_mm(which, wi, c):
                    def f():
                        if c == 0:
                            st[which] = ps_p.tile([H, QS], F32, tag="proj", name=which)
                        nc.tensor.matmul(
                            st[which][:], w_sb[:, wi, c, :], st["xt"][:, c, :],
                            start=(c == 0), stop=(c == CB - 1),
                        )
                    return f

                def k_bias():
                    nc.vector.tensor_scalar_add(
                        kT_sb[:H, cols], st["kps"][:], b_sb[:, 1:2]
                    )
                    nc.sync.dma_start(kT_sb[H:, cols], kT_sb[:H, cols])

                def v_bias():
                    nc.vector.tensor_scalar_add(
                        vTb_sb[:, cols], st["vps"][:], b_sb[:, 2:3]
                    )

                def mk_vtr(s):
                    def f():
                        tk = tb * (QS // P) + s
                        vtp = ps_t.tile([P, H], BF16, tag="tr")
                        nc.tensor.transpose(
                            vtp[:], vTb_sb[:, tk * P : (tk + 1) * P],
                            id_bf16[:H, :H],
                        )
                        nc.vector.tensor_copy(vext_sb[:, tk, :H], vtp[:])
                    return f

                th = [load]
                th += [mk_mm("kps", 1, c) for c in range(CB)]
                th += [k_bias]
                th += [mk_mm("vps", 2, c) for c in range(CB)]
                th += [v_bias]
                th += [mk_vtr(s) for s in range(QS // P)]
                return th

            def q_proj(qb):
                xq = xio.tile([P, CB, QS], BF16, tag="xt")
                nc.sync.dma_start(xq[:], x_q_d[qb])
                qps = ps_p.tile([H, QS], F32, tag="proj")
                for c in range(CB):
                    nc.tensor.matmul(
                        qps[:], w_sb[:, 0, c, :], xq[:, c, :],
                        start=(c == 0), stop=(c == CB - 1),
                    )
                qcols_ = slice(qb * QS, (qb + 1) * QS)
                nc.vector.tensor_scalar_add(qT_sb[:H, qcols_], qps[:], b_sb[:, 0:1])
                nc.sync.dma_start(qT_sb[H:, qcols_], qT_sb[:H, qcols_])

            # keys 0..1023 must exist before slot 0 attention starts
            for th in kv_thunks(0) + kv_thunks(1):
                th()

            for slot in range(NSLOT):
                U = SLOT_U[slot]
                qcols = slice(slot * QS, (slot + 1) * QS)
                q_proj(slot)
                # kv blocks for the NEXT slot, interleaved into this one
                fill = []
                if slot < NSLOT - 1:
                    fill = kv_thunks(2 * slot + 2) + kv_thunks(2 * slot + 3)
                fi = 0

                oacc = ps_o.tile([HE, QS], F32, tag="outT")
                # causal masks for this slot's diagonal band (tks U-8..U-1),
                # built in one DVE op before the attention loop needs them
                mask8 = mpool.tile([P, 8, QS], BF16, tag="mask8")
                nc.vector.tensor_tensor(
                    mask8[:],
                    qidx_sb[:, qcols][:, None, :].to_broadcast((P, 8, QS)),
                    kidx_sb[:, U - 8 : U][:, :, None].to_broadcast((P, 8, QS)),
                    mybir.AluOpType.is_ge,
                )
                pipe = []  # (expT, tkp) awaiting their wv matmuls

                def emit_wv(expT, tkp):
                    for h in range(2):
                        tk = 2 * tkp + h
                        nc.tensor.matmul(
                            oacc[:], vext_sb[:, tk, :],
                            expT[:, h, :],
                            start=(tk == 0), stop=(tk == U - 1),
                        )

                npairs = U // 2
                for tkp in range(npairs):
                    sps = ps_s.tile([P, 2, QS], F32, tag="sT")
                    expT = work.tile([P, 2, QS], BF16, tag="expT")
                    for h in range(2):
                        tk = 2 * tkp + h
                        pb = h * H  # partition base: row-groups 0-63 / 64-127
                        nc.tensor.matmul(
                            sps[:, h, :],
                            kT_sb[pb : pb + H, tk * P : (tk + 1) * P],
                            qT_sb[pb : pb + H, qcols], start=True, stop=True,
                        )
                    nc.scalar.activation(
                        expT[:], sps[:], mybir.ActivationFunctionType.Exp,
                        scale=float(H) ** -0.5,
                    )
                    if 2 * tkp >= U - 8:
                        j = 2 * tkp - (U - 8)
                        nc.vector.tensor_tensor(
                            expT[:], expT[:], mask8[:, j : j + 2, :],
                            mybir.AluOpType.mult,
                        )
                    # spread the next slot's kv projections across this slot
                    want = ((tkp + 1) * len(fill) + npairs - 1) // npairs
                    while fi < min(want, len(fill)):
                        fill[fi]()
                        fi += 1
                    # wv runs one pair behind scores so PE never stalls on ACT
                    pipe.append((expT, tkp))
                    if len(pipe) > 1:
                        emit_wv(*pipe.pop(0))
                while fi < len(fill):
                    fill[fi]()
                    fi += 1
                while pipe:
                    emit_wv(*pipe.pop(0))

                oT_sb = work.tile([HE, QS], F32, tag="oT")
                nc.vector.tensor_copy(oT_sb[:], oacc[:])
                for s in range(QS // P):
                    trp = ps_t.tile([P, HE], F32, tag="tr")
                    nc.tensor.transpose(
                        trp[:], oT_sb[:, s * P : (s + 1) * P], id_f32[:HE, :HE]
                    )
                    rec = work.tile([P, 1], F32, tag="rec")
                    nc.vector.reciprocal(rec[:], trp[:, H : H + 1])
                    ofin = work.tile([P, H], F32, tag="ofin")
                    nc.vector.tensor_scalar_mul(ofin[:], trp[:, :H], rec[:])
                    row0 = slot * QS + s * P
                    nc.sync.dma_start(out_d[row0 : row0 + P, :], ofin[:])

    nc.compile()
    return nc


_NC_CACHE = None


def _get_nc():
    global _NC_CACHE
    if _NC_CACHE is None:
        _NC_CACHE = build_bass()
    return _NC_CACHE


def _tile_xT(xT_cols):
    """[C, N*512] f32 -> [N, 128, 8, 512] bf16 pre-tiled for contiguous DMA."""
    n = xT_cols.shape[1] // QS
    t = xT_cols.reshape(CB, P, n, QS)          # [co, p, tb, t]
    return np.ascontiguousarray(t.transpose(2, 1, 0, 3).astype(BF16NP))


def _core_inputs(x, Wq, bq, Wk, bk, Wv, bv, b, fold):
    xT = np.asarray(x[b], dtype=np.float32).T  # [C, T] (view)
    qstarts = FOLD_SLOT_QSTART[fold]
    qcols = np.concatenate([np.arange(q0, q0 + QS) for q0 in qstarts])
    w_all = np.stack(
        [np.asarray(w, np.float32).reshape(CB, P, H) for w in (Wq, Wk, Wv)], axis=1
    )  # [co, 3, p, h]
    w_all = np.ascontiguousarray(w_all.transpose(2, 1, 0, 3).astype(BF16NP))
    b_all = np.ascontiguousarray(
        np.stack([np.asarray(v, np.float32) for v in (bq, bk, bv)], axis=1)
    )
    qidx = np.broadcast_to(qcols.astype(np.float32)[None, :], (P, NSLOT * QS))
    kidx = (
        np.arange(NB, dtype=np.float32)[None, :] * P
        + np.arange(P, dtype=np.float32)[:, None]
    )
    qkidx = np.ascontiguousarray(
        np.concatenate([qidx, kidx], axis=1, dtype=np.float32)
    )
    return {
        "x_kv": _tile_xT(xT),
        "x_q": _tile_xT(xT[:, qcols]),
        "w_all": w_all,
        "b_all": b_all,
        "qkidx": qkidx,
    }


def kernel(x, Wq, bq, Wk, bk, Wv, bv):
    x = np.asarray(x, dtype=np.float32)
    nc = _get_nc()
    core_ids = list(range(8))
    in_maps = [
        _core_inputs(x, Wq, bq, Wk, bk, Wv, bv, core // 2, core % 2)
        for core in core_ids
    ]
    res = run_bass_kernel_spmd(nc, in_maps, core_ids)
    out = np.empty((B, T, H), dtype=np.float32)
    for core in core_ids:
        b, fold = core // 2, core % 2
        co = res.results[core]["out"]  # [2048, 64]
        for slot, q0 in enumerate(FOLD_SLOT_QSTART[fold]):
            out[b, q0 : q0 + QS, :] = co[slot * QS : (slot + 1) * QS, :]
    return out



# revision 7
# speedup vs baseline: 1.5463x; 1.5463x over previous
"""Single-head causal attention (B=4, T=4096, C=1024, H=64) on 8 TRN2 cores.

Sharding: 2 cores per batch element, query rows split between the pair for
causal balance. Key-block EMISSION order is a per-fold permutation chosen so
that the q-block of attention slot s sits at emission position 2s on BOTH
folds; the q projection then piggybacks on the k/v projection of that block
(same x tile, no separate x_q DMA). Slot s attends emission blocks 0..2s+1:
blocks below the band are fully causal-valid by construction, emission block
2s is the true diagonal (static triangle mask, additive pre-exp), and block
2s+1 is a filler that is fully valid or fully dead per fold - zeroed for free
via a data-driven per-partition bias on the exp activation (exp(x/8 - 100)).

K and V projections are packed into one matmul ([Wv | Wk] stationary, 128
wide): v^T lands on PSUM partitions 0-63, k^T on 64-127, one bias-add writes
both into a combined vkT tile. Scores run as row-tiled pairs (PE rows 0-63 /
64-127 concurrently). The weights@V matmul uses fp8e4 DoubleRow (two key
chunks contracted per pass, K=256 virtual). A ones-column on v gives the
softmax denominator; normalization and the final transpose happen on host.
"""

import numpy as np
import ml_dtypes

import concourse.bacc as bacc
import concourse.mybir as mybir
from concourse.tile import TileContext
from concourse.masks import make_identity
from concourse.bass_utils import run_bass_kernel_spmd

B, T, C, H = 4, 4096, 1024, 64
P = 128                     # SBUF partitions
NB = T // P                 # 32 key chunks of 128
CB = C // P                 # 8 contraction chunks of 128
QS = 512                    # query/projection block width
TB = T // QS                # 8 key-side projection blocks
NSLOT = 4                   # query slots per core (2048 queries)
HE = H + 1                  # v extended with a ones column (softmax denom)
VP = 80                     # vext pair-stride padding (fp8 step % 16 == 0)

USE_DR = True               # fp8 DoubleRow for the weights@V matmul

# emission position -> key block; q-block of slot s is at position 2s on
# both folds, and position 2s+1 is the other fold's q-block (the filler).
EM_PERM = [
    [1, 0, 2, 3, 4, 5, 7, 6],   # fold 0
    [0, 1, 3, 2, 5, 4, 6, 7],   # fold 1
]
FOLD_SLOT_QSTART = [[p[2 * s] * QS for s in range(NSLOT)] for p in EM_PERM]

F32 = mybir.dt.float32
BF16 = mybir.dt.bfloat16
FP8 = mybir.dt.float8e4
EDT = FP8 if USE_DR else BF16
BF16NP = ml_dtypes.bfloat16


def build_bass():
    nc = bacc.Bacc("TRN2", target_bir_lowering=False, debug=False)

    x_d = nc.declare_dram_parameter("x_all", [TB, P, CB, QS], BF16, isOutput=False)
    wvk_d = nc.declare_dram_parameter("w_vk", [P, CB, P], BF16, isOutput=False)
    wq_d = nc.declare_dram_parameter("w_q", [P, CB, H], BF16, isOutput=False)
    # col 0: [bv; bk], col 1: [bq; 0], cols 2-5: per-slot filler exp-bias
    prm_d = nc.declare_dram_parameter("prm", [P, 8], F32, isOutput=False)
    out_d = nc.declare_dram_parameter("out", [NSLOT, HE, QS], F32, isOutput=True)

    with TileContext(nc) as tc:
        with (
            tc.tile_pool(name="const", bufs=1) as const,
            tc.tile_pool(name="xio", bufs=3) as xio,
            tc.tile_pool(name="work", bufs=3) as work,
            tc.tile_pool(name="wout", bufs=2) as wout,
            tc.tile_pool(name="ps_s", bufs=2, space="PSUM") as ps_s,
            tc.tile_pool(name="ps_o", bufs=2, space="PSUM") as ps_o,
            tc.tile_pool(name="ps_p", bufs=1, space="PSUM") as ps_p,
            tc.tile_pool(name="ps_t", bufs=1, space="PSUM") as ps_t,
        ):
            # ---- persistent SBUF state; DMA order = head critical path ----
            wvk_sb = const.tile([P, CB, P], BF16, tag="wvk")
            nc.sync.dma_start(wvk_sb[:], wvk_d[:])
            wq_sb = const.tile([P, CB, H], BF16, tag="wq")
            nc.sync.dma_start(wq_sb[:], wq_d[:])
            prm_sb = const.tile([P, 8], F32, tag="prm")
            nc.sync.dma_start(prm_sb[:], prm_d[:])

            vkT = const.tile([P, T], BF16, tag="vkT")      # v^T low / k^T high
            kTlow = const.tile([H, T], BF16, tag="kTl")    # k^T copy, parts 0-63
            qTd = const.tile([P, NSLOT * QS], BF16, tag="qTd")  # q^T dup halves
            vext = const.tile([P, NB // 2, 2, VP], EDT, tag="vext")
            nc.vector.memset(vext[:, :, :, H:HE], 1.0)
            # slot 0 queries attend few keys: fp8 v-quantization doesn't
            # average out there, so slot 0 runs bf16 wv on chunks 0-7
            if USE_DR:
                vext_bf = const.tile([P, 4, 2, VP], BF16, tag="vextbf")
                nc.vector.memset(vext_bf[:, :, :, H:HE], 1.0)
            else:
                vext_bf = vext

            id64 = const.tile([H, H], BF16, tag="id64")
            make_identity(nc, id64[:])

            # static causal triangle for the diagonal 512-block (additive,
            # pre-exp): tri[p, c, qi] = 0 if qi >= c*128 + p else -1e4
            tri = const.tile([P, 4, QS], F32, tag="tri")
            nc.gpsimd.memset(tri[:], 0.0)
            for c in range(4):
                nc.gpsimd.affine_select(
                    out=tri[:, c, :], in_=tri[:, c, :],
                    compare_op=mybir.AluOpType.is_ge,
                    fill=-1e4, base=-c * P,
                    pattern=[[1, QS]], channel_multiplier=-1,
                )

            # ---- emission thunks (PE is in-order; emission order is the
            # static schedule). Each key block: 2 half DMAs, 8 packed [Wv|Wk]
            # matmuls, one bias-add writing v^T+k^T, k^T low copy, optional q
            # projection (emission positions 0,2,4,6), 4 v transposes. ----
            def kv_thunks(em):
                st = {}
                cols = slice(em * QS, (em + 1) * QS)

                def mk_load(hf):
                    def f():
                        if hf == 0:
                            st["xt"] = xio.tile([P, CB, QS], BF16, tag="xt", name="xt")
                        nc.sync.dma_start(
                            st["xt"][:, 4 * hf : 4 * hf + 4, :],
                            x_d[em, :, 4 * hf : 4 * hf + 4, :],
                        )
                    return f

                def mk_mm(c):
                    def f():
                        if c == 0:
                            st["vk"] = ps_p.tile([P, QS], F32, tag="pp", name="vk")
                        nc.tensor.matmul(
                            st["vk"][:], wvk_sb[:, c, :], st["xt"][:, c, :],
                            start=(c == 0), stop=(c == CB - 1),
                        )
                    return f

                def bias():
                    nc.vector.tensor_scalar_add(
                        vkT[:, cols], st["vk"][:], prm_sb[:, 0:1]
                    )

                def ktcopy():
                    nc.sync.dma_start(kTlow[:, cols], vkT[H:, cols])

                def mk_qmm(c):
                    def f():
                        if c == 0:
                            st["q"] = ps_p.tile([H, QS], F32, tag="pp", name="q")
                        nc.tensor.matmul(
                            st["q"][:], wq_sb[:, c, :], st["xt"][:, c, :],
                            start=(c == 0), stop=(c == CB - 1),
                        )
                    return f

                qcols = slice((em // 2) * QS, (em // 2 + 1) * QS)

                def qbias():
                    nc.vector.tensor_scalar_add(
                        qTd[:H, qcols], st["q"][:], prm_sb[:H, 1:2]
                    )

                def qcopy():
                    nc.sync.dma_start(qTd[H:, qcols], qTd[:H, qcols])

                def mk_vtr(s):
                    def f():
                        tk = 4 * em + s
                        vtp = ps_t.tile([P, H], BF16, tag="tr")
                        nc.tensor.transpose(
                            vtp[:], vkT[:H, tk * P : (tk + 1) * P], id64[:]
                        )
                        nc.vector.tensor_copy(
                            vext[:, tk // 2, tk % 2, :H], vtp[:]
                        )
                        if USE_DR and em < 2:
                            nc.vector.tensor_copy(
                                vext_bf[:, tk // 2, tk % 2, :H], vtp[:]
                            )
                    return f

                th = [mk_load(0), mk_load(1)]
                th += [mk_mm(c) for c in range(CB)]
                th += [bias, ktcopy]
                if em % 2 == 0:
                    th += [mk_qmm(c) for c in range(CB)]
                    th += [qbias, qcopy]
                th += [mk_vtr(s) for s in range(4)]
                return th

            # keys/q of emission blocks 0,1 must exist before slot 0 starts
            for th in kv_thunks(0) + kv_thunks(1):
                th()

            for slot in range(NSLOT):
                npairs = 4 * (slot + 1)
                qcols = slice(slot * QS, (slot + 1) * QS)
                fill = []
                if slot < NSLOT - 1:
                    fill = kv_thunks(2 * slot + 2) + kv_thunks(2 * slot + 3)
                fi = 0

                oacc = ps_o.tile([HE, QS], F32, tag="oacc")
                pipe = []  # expT pairs awaiting their wv matmul
                use_dr = USE_DR and slot > 0

                def emit_wv(expT, tkp, npairs=npairs, oacc=oacc, use_dr=use_dr):
                    if use_dr:
                        nc.tensor.matmul(
                            oacc[:], vext[:, tkp, :, :HE], expT[:],
                            start=(tkp == 0), stop=(tkp == npairs - 1),
                            perf_mode=mybir.MatmulPerfMode.DoubleRow,
                        )
                    else:
                        for h in range(2):
                            tk = 2 * tkp + h
                            nc.tensor.matmul(
                                oacc[:], vext_bf[:, tkp, h, :HE], expT[:, h, :],
                                start=(tk == 0), stop=(tk == 2 * npairs - 1),
                            )

                for tkp in range(npairs):
                    sps = ps_s.tile([P, 2, QS], F32, tag="sT")
                    for h in range(2):
                        tk = 2 * tkp + h
                        if h == 0:
                            nc.tensor.matmul(
                                sps[:, 0, :], kTlow[:, tk * P : (tk + 1) * P],
                                qTd[:H, qcols], start=True, stop=True,
                            )
                        else:
                            nc.tensor.matmul(
                                sps[:, 1, :], vkT[H:, tk * P : (tk + 1) * P],
                                qTd[H:, qcols], start=True, stop=True,
                            )
                    j = tkp - 4 * slot
                    if j in (0, 1):  # true diagonal block: triangle mask
                        nc.vector.tensor_tensor(
                            sps[:], sps[:], tri[:, 2 * j : 2 * j + 2, :],
                            mybir.AluOpType.add,
                        )
                    expT = work.tile([P, 2, QS], EDT if use_dr else BF16,
                                     tag="expT")
                    # filler block (j in 2,3): dead folds zeroed via exp bias
                    fb = prm_sb[:, 2 + slot : 3 + slot] if j in (2, 3) else 0.0
                    nc.scalar.activation(
                        expT[:], sps[:], mybir.ActivationFunctionType.Exp,
                        scale=float(H) ** -0.5, bias=fb,
                    )
                    # spread the next blocks' projections across this slot
                    want = ((tkp + 1) * len(fill) + npairs - 1) // npairs
                    while fi < min(want, len(fill)):
                        fill[fi]()
                        fi += 1
                    # wv runs one pair behind scores so PE never stalls on ACT
                    pipe.append((expT, tkp))
                    if len(pipe) > 1:
                        emit_wv(*pipe.pop(0))
                while fi < len(fill):
                    fill[fi]()
                    fi += 1
                while pipe:
                    emit_wv(*pipe.pop(0))

                oT = wout.tile([HE, QS], F32, tag="oT")
                nc.vector.tensor_copy(oT[:], oacc[:])
                nc.sync.dma_start(out_d[slot], oT[:])

    nc.compile()
    return nc


_NC_CACHE = None


def _get_nc():
    global _NC_CACHE
    if _NC_CACHE is None:
        _NC_CACHE = build_bass()
    return _NC_CACHE


def _core_inputs(x, Wq, bq, Wk, bk, Wv, bv, b, fold):
    xT = np.asarray(x[b], dtype=np.float32).T       # [C, T] view
    perm = EM_PERM[fold]
    xa = np.empty((TB, P, CB, QS), dtype=BF16NP)
    for em, blk in enumerate(perm):
        t = xT[:, blk * QS : (blk + 1) * QS].reshape(CB, P, QS)
        xa[em] = t.transpose(1, 0, 2)

    wv = np.asarray(Wv, np.float32).reshape(CB, P, H)
    wk = np.asarray(Wk, np.float32).reshape(CB, P, H)
    w_vk = np.concatenate([wv, wk], axis=2).transpose(1, 0, 2)  # [P, CB, 128]
    w_q = np.asarray(Wq, np.float32).reshape(CB, P, H).transpose(1, 0, 2)

    prm = np.zeros((P, 8), dtype=np.float32)
    prm[:H, 0] = np.asarray(bv, np.float32)
    prm[H:, 0] = np.asarray(bk, np.float32)
    prm[:H, 1] = np.asarray(bq, np.float32)
    for s in range(NSLOT):
        if perm[2 * s + 1] > perm[2 * s]:   # filler block is fully dead
            prm[:, 2 + s] = -100.0
    return {
        "x_all": np.ascontiguousarray(xa),
        "w_vk": np.ascontiguousarray(w_vk.astype(BF16NP)),
        "w_q": np.ascontiguousarray(w_q.astype(BF16NP)),
        "prm": prm,
    }


def _unshard(results):
    out = np.empty((B, T, H), dtype=np.float32)
    for core in range(8):
        b, fold = core // 2, core % 2
        o = results[core]["out"]            # [NSLOT, 65, 512]
        for s, q0 in enumerate(FOLD_SLOT_QSTART[fold]):
            out[b, q0 : q0 + QS, :] = (o[s, :H, :] / o[s, H : H + 1, :]).T
    return out


def kernel(x, Wq, bq, Wk, bk, Wv, bv):
    x = np.asarray(x, dtype=np.float32)
    nc = _get_nc()
    core_ids = list(range(8))
    in_maps = [
        _core_inputs(x, Wq, bq, Wk, bk, Wv, bv, core // 2, core % 2)
        for core in core_ids
    ]
    res = run_bass_kernel_spmd(nc, in_maps, core_ids)
    return _unshard(res.results)
